# revision 45
# baseline (speedup 1.0000x reference)
"""MLA forward kernel for Trainium2, 8 NeuronCores.

Sharding: 2 batch groups x 4 head groups. Core c handles batch b=c//4 and
heads 4g..4g+3 where g=c%4. The LoRA down-projections (P1) are token-sharded
within each batch group: core (b, g) computes q/kv lora + layernorm + k-rope
rotation for token quarter g only, then two AllGathers (kv first, then q)
rebuild the full-T activations on every core. Attention and the partial
output projection stay head-sharded as before; the host sums 4 partials per
batch and adds the output bias.

Matmuls run in bf16 (fp32 PSUM accumulation), except the q up-projection
which runs in fp8e4m3 with DoubleRow pairing (two 128-row K planes per
instruction, 2x PE throughput): qln and wq_b are the two least
error-sensitive operands (LN bounds qln; softmax normalization absorbs
common-mode score error), measured output rel-err 8.7e-3 vs the 2e-2 gate.
Layout is feature-major (features on partitions, tokens on free dim).
RoPE rotate-half is a PE permutation matmul with the rotation signs folded
into the host-precomputed sin table. Causal softmax runs without max
subtraction; exp row-sums come from the scalar engine's accum_out.

Schedule notes: kv-lora runs before q-lora so its AllGather flies under the
q matmuls; P2 weight loads ride the scalar-DMA queue during P1; the P4
weight load is prefetched during attention. The "dep"/"dep_out" passthrough
tensors let the timing harness serially chain kernel executions.
"""
import sys

sys.path.insert(0, "/opt/trn_rl_repo")

import math
from contextlib import ExitStack

import numpy as np
import ml_dtypes

import concourse.bacc as bacc
import concourse.bass as bass
import concourse.tile as tile
from concourse import mybir
from concourse.bass_utils import run_bass_kernel_spmd
from concourse.masks import make_identity

F32 = mybir.dt.float32
BF16 = mybir.dt.bfloat16
FP8 = mybir.dt.float8e4
AF = mybir.ActivationFunctionType
ALU = mybir.AluOpType
DR_MODE = mybir.MatmulPerfMode.DoubleRow
BF = ml_dtypes.bfloat16
F8 = ml_dtypes.float8_e4m3
WQB_SCALE = 64.0   # wq_b is shipped as fp8 * WQB_SCALE; undone at PSUM readout

B, T, DIM = 2, 2048, 2048
H, QLR, KVLR = 16, 1024, 512
DN, DR, DV = 128, 64, 128
DQK = DN + DR
EPS = 1e-5
HPG = 4          # heads per group (per core)
NCORES = 8
SCALE = 1.0 / math.sqrt(DQK)
NT = T // 512    # 512-wide token tiles
NQT = T // 128   # 128-row query tiles
MASK_NEG = -1e30
RG = [[0, 1, 2, 3], [4, 5, 6, 7]]   # batch groups = AllGather replica groups

_cached = {}


def _ts(i, n):
    return slice(i * n, (i + 1) * n)


def build_bass():
    nc = bacc.Bacc("TRN2", target_bir_lowering=False, debug=False, num_devices=NCORES)

    inp = {}
    def di(name, shape, dt):
        inp[name] = nc.dram_tensor(name, list(shape), dt, kind="ExternalInput")
        return inp[name]

    # big tensors are flat (128, N) with per-partition-contiguous layout so
    # every load is 128 single-run descriptors
    di("xt", (128, 16 * 512), BF16)       # x[b].T quarter, chunked (p, cc, t)
    di("wqa", (128, 16 * QLR), BF16)      # wq_a.T chunked (p=c, cc, l)
    di("wkva", (128, 16 * (KVLR + DR)), BF16)
    di("wqbn", (128, 8 * HPG * DN), FP8)   # nope rows of wq_b (group), .T chunked by l
    di("wqbr", (128, 8 * HPG * DR), FP8)   # rope rows
    di("wkvbk", (128, 4 * HPG * DN), BF16)
    di("wkvbv", (128, 4 * HPG * DV), BF16)  # moving operand (p=lc, lc, hd)
    di("wout_l", (128, HPG * DIM), BF16)   # lhsT (p=hd within head, head, o)
    di("cosq", (128, T), BF16)             # [cos32;cos32] stacked twice (head pair)
    di("sinqs", (128, T), BF16)            # [-sin32;+sin32] stacked twice
    di("cosq_loc", (64, 512), BF16)        # this core's token-quarter columns
    di("sinqs_loc", (64, 512), BF16)
    di("perm64", (64, 64), BF16)           # rotate-half swap lhsT
    di("perm128", (128, 128), BF16)        # block-diag pair version
    di("maskt", (128, 4, 512), F32)        # additive causal masks, variant v=qt%4
    di("dep", (128, 16), F32)             # chain-dependency token (timing harness)
    di("bqa_t", (128, 8), F32)
    di("gq_t", (128, 8), F32)
    di("bq_t", (128, 8), F32)
    di("bqbn_t", (128, HPG), F32)
    di("bqbr_t", (128, 2), F32)
    di("bkva_t", (128, 5), F32)            # 576 rows chunked, last chunk rows 0:64
    di("bkvbk_t", (128, HPG), F32)
    di("bkvbv_row", (1, HPG * DV), F32)    # v bias as row (broadcast over partitions)

    outp = nc.dram_tensor("outp", [DIM, T], BF16, kind="ExternalOutput")
    dep_out = nc.dram_tensor("dep_out", [128, 16], F32, kind="ExternalOutput")

    with tile.TileContext(nc) as tc, ExitStack() as es:
        cst = es.enter_context(tc.tile_pool(name="cst", bufs=1))
        dram = es.enter_context(tc.tile_pool(name="dram", bufs=1, space="DRAM"))
        pD = es.enter_context(tc.tile_pool(name="pD", bufs=1))    # qln, kvl, kr (P1->P2)

        # ---- small constants (live whole kernel) ----
        ones_bf = cst.tile([128, 1], BF16)
        nc.vector.memset(ones_bf[:], 1.0)
        eps_t = cst.tile([1, 1], F32)
        nc.vector.memset(eps_t[:], EPS)
        # small constants ride the gpsimd queue so the sync queue starts with
        # the x tiles and the scalar queue with the P1 weights immediately
        perm = cst.tile([64, 64], BF16)
        nc.gpsimd.dma_start(out=perm[:], in_=inp["perm64"][:, :])
        cosq_loc = cst.tile([64, 512], BF16)
        nc.gpsimd.dma_start(out=cosq_loc[:], in_=inp["cosq_loc"][:, :])
        sinqs_loc = cst.tile([64, 512], BF16)
        nc.gpsimd.dma_start(out=sinqs_loc[:], in_=inp["sinqs_loc"][:, :])
        dep_t = cst.tile([128, 16], F32)
        nc.gpsimd.dma_start(out=dep_t[:], in_=inp["dep"][:, :])
        nc.gpsimd.dma_start(out=dep_out[:, :], in_=dep_t[:])
        bias_t = {}
        for nm, shape in [("bqa_t", (128, 8)), ("gq_t", (128, 8)), ("bq_t", (128, 8)),
                          ("bqbn_t", (128, HPG)), ("bqbr_t", (128, 2)),
                          ("bkva_t", (128, 5)), ("bkvbk_t", (128, HPG))]:
            bias_t[nm] = cst.tile(list(shape), F32, tag=nm, name=nm)
            nc.gpsimd.dma_start(out=bias_t[nm][:], in_=inp[nm][:, :])

        # ---- persistent full-T intermediates (gathered; P1 -> P2) ----
        qln = pD.tile([128, 8, T], FP8)       # layernormed q_lora (fp8), full T
        kvl = pD.tile([128, 4, T], BF16)      # kv_lora, full T
        kr = pD.tile([128, T], BF16)          # rotated k rope, duplicated halves

        # single AllGather bounce buffer (one rendezvous per iteration);
        # rank g's chunk = rows [128g:128(g+1)): cols 0:4 kvl, 4 kr, 5:13 qln
        ag_in = dram.tile([128, 13, 512], BF16)
        ag_out = dram.tile([4 * 128, 13, 512], BF16)

        # P2 weights/tables: loaded on the scalar queue right after the P1
        # weights so P2 never waits on them
        pW = es.enter_context(tc.tile_pool(name="pW", bufs=1))
        wkvbv = pW.tile([128, 4, HPG * DV], BF16)
        vb_bc = pW.tile([128, HPG * DV], F32)
        wkvbk = pW.tile([128, 4, HPG * DN], BF16)
        cosq = pW.tile([128, T], BF16)     # two stacked 64-row head blocks
        sinqs = pW.tile([128, T], BF16)
        wqbn = pW.tile([128, 8, HPG * DN], FP8)
        wqbr = pW.tile([128, 8, HPG * DR], FP8)

        # ================= P1: LoRA projections (token quarter only) =========
        with tc.tile_pool(name="w1", bufs=1) as w1, \
             tc.tile_pool(name="xpa", bufs=1) as xpa, \
             tc.tile_pool(name="p1loc", bufs=1) as p1loc, \
             tc.tile_pool(name="p1e", bufs=2) as p1e, \
             tc.tile_pool(name="bcps", bufs=1, space="PSUM") as bcps_pool, \
             tc.tile_pool(name="p1ps", bufs=4, space="PSUM") as p1ps, \
             tc.tile_pool(name="stps", bufs=1, space="PSUM") as stps:
            # kv weights first: the kv-lora matmuls run first so their
            # AllGather can fly under the q-lora matmuls
            wkva = w1.tile([128, 16, KVLR + DR], BF16)
            nc.scalar.dma_start(out=wkva[:, 0:2, :], in_=inp["wkva"][:, 0:2 * (KVLR + DR)])
            nc.scalar.dma_start(out=wkva[:, 2:4, :],
                                in_=inp["wkva"][:, 2 * (KVLR + DR):4 * (KVLR + DR)])
            for c4 in range(1, 4):
                nc.scalar.dma_start(out=wkva[:, _ts(c4, 4), :],
                                    in_=inp["wkva"][:, c4 * 4 * (KVLR + DR):(c4 + 1) * 4 * (KVLR + DR)])
            wqa = w1.tile([128, 16, QLR], BF16)
            nc.scalar.dma_start(out=wqa[:], in_=inp["wqa"][:, :])
            # P2 weights follow on the same queue
            nc.scalar.dma_start(out=wkvbv[:, :, :], in_=inp["wkvbv"][:, :])
            nc.scalar.dma_start(out=vb_bc[:], in_=inp["bkvbv_row"][:, :].to_broadcast([128, HPG * DV]))
            nc.scalar.dma_start(out=wkvbk[:, :, :], in_=inp["wkvbk"][:, :])
            nc.scalar.dma_start(out=cosq[:], in_=inp["cosq"][:, :])
            nc.scalar.dma_start(out=sinqs[:], in_=inp["sinqs"][:, :])
            nc.scalar.dma_start(out=wqbn[:, :, :], in_=inp["wqbn"][:, :])
            nc.scalar.dma_start(out=wqbr[:, :, :], in_=inp["wqbr"][:, :])
            ones_row = cst.tile([1, 128], BF16, tag="ones_row", name="ones_row")
            nc.vector.memset(ones_row[:], 1.0)

            xtile = xpa.tile([128, 16, 512], BF16)
            nc.sync.dma_start(out=xtile[:, 0:2, :], in_=inp["xt"][:, 0:2 * 512])
            nc.sync.dma_start(out=xtile[:, 2:4, :], in_=inp["xt"][:, 2 * 512:4 * 512])
            for c4 in range(1, 4):
                nc.sync.dma_start(out=xtile[:, _ts(c4, 4), :],
                                  in_=inp["xt"][:, c4 * 4 * 512:(c4 + 1) * 4 * 512])

            # local (this quarter) staging tiles
            kvl_loc = p1loc.tile([128, 4, 512], BF16)
            kr_loc = p1loc.tile([64, 512], BF16)
            qloc = p1loc.tile([128, 8, 512], BF16)

            # ---- kv-lora in two PSUM waves (3 + 2 accumulators) ----
            for ocs in ([0, 1, 2], [3, 4]):
                pss = [(oc, p1ps.tile([128, 512], F32, tag="p1ps", name="ps"))
                       for oc in ocs]
                for cc in range(16):
                    for oc, ps in pss:
                        rows_n = 128 if oc < 4 else 64
                        nc.tensor.matmul(ps[:rows_n, :],
                                         wkva[:, cc, oc * 128:oc * 128 + rows_n],
                                         xtile[:, cc, :], start=(cc == 0), stop=(cc == 15))
                for oc, ps in pss:
                    if oc < 4:
                        nc.scalar.activation(out=kvl_loc[:, oc, :], in_=ps[:], func=AF.Identity,
                                             bias=bias_t["bkva_t"][:, oc:oc + 1])
                    else:
                        # decoupled k-rope: rotate locally (tables are this
                        # quarter's columns), pre-gather
                        kraw = p1e.tile([64, 512], BF16, tag="kraw", name="kraw")
                        nc.scalar.activation(out=kraw[:], in_=ps[:64, :], func=AF.Identity,
                                             bias=bias_t["bkva_t"][0:64, 4:5])
                        sw = p1ps.tile([128, 512], F32, tag="p1ps", name="sw")
                        nc.tensor.matmul(sw[:64, :], perm[:], kraw[:], start=True, stop=True)
                        ta = p1e.tile([64, 512], F32, tag="ropea", name="ta")
                        nc.vector.tensor_mul(ta[:], kraw[:], cosq_loc[:])
                        tb = p1e.tile([64, 512], F32, tag="ropeb", name="tb")
                        nc.vector.tensor_mul(tb[:], sw[:64, :], sinqs_loc[:])
                        nc.vector.tensor_add(kr_loc[:], ta[:], tb[:])
            # stage kv+kr into the merged AllGather buffer
            nc.sync.dma_start(out=ag_in[:, 0:4, :], in_=kvl_loc[:, :, :])
            nc.sync.dma_start(out=ag_in[0:64, 4, :], in_=kr_loc[:])

            # ---- q-lora + LN (local quarter) ----
            stats = stps.tile([1, 1024], F32)

            for lc in range(8):
                ps = p1ps.tile([128, 512], F32, tag="p1ps")
                for cc in range(16):
                    nc.tensor.matmul(ps[:], wqa[:, cc, _ts(lc, 128)], xtile[:, cc, :],
                                     start=(cc == 0), stop=(cc == 15))
                nc.scalar.activation(out=qloc[:, lc, :], in_=ps[:], func=AF.Identity,
                                     bias=bias_t["bqa_t"][:, lc:lc + 1])
                sq = p1e.tile([128, 512], BF16, tag="sq")
                nc.vector.tensor_mul(sq[:], qloc[:, lc, :], qloc[:, lc, :])
                nc.tensor.matmul(stats[:, 0:512], ones_bf[:], qloc[:, lc, :],
                                 start=(lc == 0), stop=(lc == 7))
                nc.tensor.matmul(stats[:, 512:1024], ones_bf[:], sq[:],
                                 start=(lc == 0), stop=(lc == 7))
            r1 = p1e.tile([1, 512], F32, tag="r1")
            r2 = p1e.tile([1, 512], F32, tag="r2")
            mrow_bf = p1e.tile([1, 512], BF16, tag="mrow_bf")
            rrow_bf = p1e.tile([1, 512], BF16, tag="rrow_bf")
            mrow_f = p1e.tile([1, 512], F32, tag="mrow_f")
            nc.vector.tensor_scalar_mul(mrow_f[:], stats[0:1, 0:512], 1.0 / QLR)
            nc.vector.tensor_scalar_mul(r1[:], stats[0:1, 512:1024], 1.0 / QLR)
            nc.vector.tensor_mul(r2[:], mrow_f[:], mrow_f[:])
            nc.vector.tensor_sub(r1[:], r1[:], r2[:])          # var
            nc.scalar.activation(out=r2[:], in_=r1[:], func=AF.Sqrt, bias=eps_t[:])
            with nc.allow_low_precision(reason="LN row broadcast via PE"):
                nc.vector.reciprocal(out=rrow_bf[:], in_=r2[:])
                nc.vector.tensor_copy(out=mrow_bf[:], in_=mrow_f[:])
            bcps = bcps_pool.tile([128, 1024], F32, tag="bc", name="bc")
            nc.tensor.matmul(bcps[:, 0:512], ones_row[:], mrow_bf[:],
                             start=True, stop=True)
            nc.tensor.matmul(bcps[:, 512:1024], ones_row[:], rrow_bf[:],
                             start=True, stop=True)
            # LN apply in place on the local staging tile
            for lc in range(8):
                t1 = p1e.tile([128, 512], BF16, tag="lnt")
                nc.vector.tensor_sub(t1[:], qloc[:, lc, :], bcps[:, 0:512])
                nc.vector.tensor_mul(t1[:], t1[:], bcps[:, 512:1024])
                nc.scalar.activation(out=qloc[:, lc, :], in_=t1[:], func=AF.Identity,
                                     scale=bias_t["gq_t"][:, lc:lc + 1],
                                     bias=bias_t["bq_t"][:, lc:lc + 1])
            # ship q chunk, kick the single AllGather
            nc.sync.dma_start(out=ag_in[:, 5:13, :], in_=qloc[:, :, :])
            nc.gpsimd.collective_compute(
                "AllGather", ALU.bypass, replica_groups=RG,
                ins=[ag_in.opt()], outs=[ag_out.opt()])

            # ---- readback: kv/kr on the scalar queue; qln via the gpsimd
            # casting DMA (bf16 transport -> fp8 tile for the DoubleRow) ----
            for tt in range(NT):
                rs = slice(tt * 128, (tt + 1) * 128)
                nc.scalar.dma_start(out=kvl[:, :, _ts(tt, 512)],
                                    in_=ag_out[rs, 0:4, :])
                nc.scalar.dma_start(out=kr[0:64, _ts(tt, 512)],
                                    in_=ag_out[tt * 128:tt * 128 + 64, 4, :])
                nc.scalar.dma_start(out=kr[64:128, _ts(tt, 512)],
                                    in_=ag_out[tt * 128:tt * 128 + 64, 4, :])
                nc.gpsimd.dma_start(out=qln[:, :, _ts(tt, 512)],
                                    in_=ag_out[rs, 5:13, :])

        # ================= P2: up-projections + rope =================
        pG = es.enter_context(tc.tile_pool(name="pG", bufs=1))    # q/k/v heads (P2->P3)
        qnope = pG.tile([128, HPG, T], BF16)
        qrope = pG.tile([128, 2, T], BF16)   # head PAIR hp: h=2hp at rows 0:64, 2hp+1 at 64:128
        knope = pG.tile([128, HPG, T], BF16)
        vtm = pG.tile([128, NQT, HPG * DV], BF16)   # V token-major (k, kt, hd)
        perm2 = pG.tile([128, 128], BF16)
        nc.gpsimd.dma_start(out=perm2[:], in_=inp["perm128"][:, :])

        with tc.tile_pool(name="p2e", bufs=4) as p2e, \
             tc.tile_pool(name="p2ps", bufs=3, space="PSUM") as p2ps, \
             tc.tile_pool(name="p2ps64", bufs=2, space="PSUM") as p2ps64:

            def rope_block2(dst_ap, src_ap, ts):
                """dst = rotate_half(src), two stacked 64-row head blocks (128, 512)."""
                sw = p2ps64.tile([128, 512], F32, tag="swap", name="sw")
                nc.tensor.matmul(sw[:], perm2[:], src_ap, start=True, stop=True)
                ta = p2e.tile([128, 512], F32, tag="ropea", name="ta")
                nc.vector.tensor_mul(ta[:], src_ap, cosq[:, ts])
                tb = p2e.tile([128, 512], F32, tag="ropeb", name="tb")
                nc.vector.tensor_mul(tb[:], sw[:], sinqs[:, ts])
                nc.vector.tensor_add(dst_ap, ta[:], tb[:])

            # kv-side first: only depends on AllGather 1
            for kt in range(NQT):
                ps = p2ps.tile([128, 512], F32, tag="p2ps", name="ps")
                for lc in range(4):
                    nc.tensor.matmul(ps[:], kvl[:, lc, _ts(kt, 128)], wkvbv[:, lc, :],
                                     start=(lc == 0), stop=(lc == 3))
                nc.vector.tensor_add(vtm[:, kt, :], ps[:], vb_bc[:])
            for h in range(HPG):
                for tt in range(NT):
                    ts = _ts(tt, 512)
                    ps = p2ps.tile([128, 512], F32, tag="p2ps", name="ps")
                    for lc in range(4):
                        nc.tensor.matmul(ps[:], wkvbk[:, lc, _ts(h, DN)], kvl[:, lc, ts],
                                         start=(lc == 0), stop=(lc == 3))
                    nc.scalar.activation(out=knope[:, h, ts], in_=ps[:], func=AF.Identity,
                                         bias=bias_t["bkvbk_t"][:, h:h + 1])
            for tt in range(NT):
                for h in range(HPG):
                    ts = _ts(tt, 512)
                    # q nope
                    ps = p2ps.tile([128, 512], F32, tag="p2ps", name="ps")
                    for lp in range(4):
                        nc.tensor.matmul(ps[:], wqbn[:, 2 * lp:2 * lp + 2, _ts(h, DN)],
                                         qln[:, 2 * lp:2 * lp + 2, ts],
                                         perf_mode=DR_MODE,
                                         start=(lp == 0), stop=(lp == 3))
                    nc.scalar.activation(out=qnope[:, h, ts], in_=ps[:], func=AF.Identity,
                                         scale=1.0 / WQB_SCALE,
                                         bias=bias_t["bqbn_t"][:, h:h + 1])
                # q rope: two heads per matmul (full 128-wide PE)
                for hp in range(2):
                    ts = _ts(tt, 512)
                    ps2 = p2ps64.tile([128, 512], F32, tag="qr", name="ps2")
                    for lp in range(4):
                        nc.tensor.matmul(ps2[:], wqbr[:, 2 * lp:2 * lp + 2, _ts(hp, 2 * DR)],
                                         qln[:, 2 * lp:2 * lp + 2, ts],
                                         perf_mode=DR_MODE,
                                         start=(lp == 0), stop=(lp == 3))
                    qr_raw = p2e.tile([128, 512], BF16, tag="qr_raw", name="qr_raw")
                    nc.scalar.activation(out=qr_raw[:], in_=ps2[:], func=AF.Identity,
                                         scale=1.0 / WQB_SCALE,
                                         bias=bias_t["bqbr_t"][:, hp:hp + 1])
                    rope_block2(qrope[:, hp, ts], qr_raw[:], ts)

        # ================= P3: causal attention =================
        pI = es.enter_context(tc.tile_pool(name="pI", bufs=1))
        yt = pI.tile([128, HPG, T], BF16)           # attention out, feature-major
        wout_l = pI.tile([128, HPG, DIM], BF16)     # P4 weights, prefetched during P3
        nc.scalar.dma_start(out=wout_l[:, :, :], in_=inp["wout_l"][:, :])
        idb = pI.tile([128, 128], BF16)
        make_identity(nc, idb[:])

        with tc.tile_pool(name="amask", bufs=1) as amask, \
             tc.tile_pool(name="ap_s", bufs=3) as ap_s, \
             tc.tile_pool(name="ap_l", bufs=4) as ap_l, \
             tc.tile_pool(name="sps", bufs=3, space="PSUM") as spsp, \
             tc.tile_pool(name="ptps", bufs=2, space="PSUM") as ptps, \
             tc.tile_pool(name="yps", bufs=2, space="PSUM") as ypsp, \
             tc.tile_pool(name="ytps", bufs=1, space="PSUM") as ytpsp:
            maskt = amask.tile([128, 4, 512], F32)
            nc.sync.dma_start(out=maskt[:], in_=inp["maskt"][:, :, :])

            for h in range(HPG):
                for qt in range(NQT):
                    nkt = qt // 4 + 1
                    qs = _ts(qt, 128)
                    yps = ypsp.tile([128, 128], F32, tag="yacc", name="yps")
                    lpart = ap_l.tile([128, 4], F32, tag="lpart", name="lpart")
                    for kt in range(nkt):
                        # diagonal tile only covers its valid key width
                        diag = kt == qt // 4
                        w = (qt % 4 + 1) * 128 if diag else 512
                        nsub = qt % 4 + 1 if diag else 4
                        ks = slice(kt * 512, kt * 512 + w)
                        sps = spsp.tile([128, 512], F32, tag="sps", name="sps")
                        hb = 64 * (h % 2)
                        nc.tensor.matmul(sps[:, :w], qnope[:, h, qs], knope[:, h, ks],
                                         start=True, stop=False)
                        nc.tensor.matmul(sps[:, :w], qrope[hb:hb + 64, h // 2, qs],
                                         kr[hb:hb + 64, ks],
                                         start=False, stop=True)
                        if diag:
                            nc.vector.tensor_add(sps[:, :w], sps[:, :w],
                                                 maskt[:, qt % 4, :w])
                        pbf = ap_s.tile([128, 512], BF16, tag="pbf", name="pbf")
                        nc.scalar.activation(out=pbf[:, :w], in_=sps[:, :w], func=AF.Exp,
                                             scale=SCALE,
                                             accum_out=lpart[:, kt:kt + 1])
                        ptp = ptps.tile([128, 512], BF16, tag="ptp", name="ptp")
                        for i in range(nsub):
                            nc.tensor.transpose(ptp[:, _ts(i, 128)], pbf[:, _ts(i, 128)], idb[:])
                        pts = ap_s.tile([128, 512], BF16, tag="pts", name="pts")
                        nc.vector.tensor_copy(out=pts[:, :w], in_=ptp[:, :w])
                        for i in range(nsub):
                            nc.tensor.matmul(yps[:], pts[:, _ts(i, 128)],
                                             vtm[:, kt * 4 + i, _ts(h, DV)],
                                             start=(kt == 0 and i == 0),
                                             stop=(kt == nkt - 1 and i == nsub - 1))
                    lsum = ap_l.tile([128, 1], F32, tag="lsum", name="lsum")
                    nc.vector.tensor_reduce(lsum[:], lpart[:, 0:nkt],
                                            axis=mybir.AxisListType.X, op=ALU.add)
                    linv = ap_l.tile([128, 1], F32, tag="linv", name="linv")
                    nc.vector.reciprocal(out=linv[:], in_=lsum[:])
                    ytmb = ap_s.tile([128, 128], BF16, tag="ytmb", name="ytmb")
                    nc.vector.tensor_scalar_mul(ytmb[:], yps[:], linv[:])
                    ytp = ytpsp.tile([128, 128], BF16, tag="ytp", name="ytp")
                    nc.tensor.transpose(ytp[:], ytmb[:], idb[:])
                    nc.vector.tensor_copy(out=yt[:, h, qs], in_=ytp[:])

        # ================= P4: output projection (partial) =================
        with tc.tile_pool(name="p4e", bufs=4) as p4e, \
             tc.tile_pool(name="p4ps", bufs=4, space="PSUM") as p4ps:
            for oc in range(16):
                for tt in range(NT):
                    ts = _ts(tt, 512)
                    ps = p4ps.tile([128, 512], F32, tag="p4ps", name="ps")
                    for h in range(HPG):
                        nc.tensor.matmul(ps[:], wout_l[:, h, _ts(oc, 128)], yt[:, h, ts],
                                         start=(h == 0), stop=(h == HPG - 1))
                    ot = p4e.tile([128, 512], BF16, tag="ot", name="ot")
                    with nc.allow_low_precision(reason="bf16 output partials"):
                        nc.scalar.copy(out=ot[:], in_=ps[:])
                    nc.sync.dma_start(out=outp[_ts(oc, 128), ts], in_=ot[:])

    nc.compile()
    return nc


def _chunk(a, p=128):
    """(N, M) -> (p, N//p, M) with chunk index as middle dim."""
    n, m = a.shape
    return np.ascontiguousarray(a.reshape(n // p, p, m).swapaxes(0, 1))


def _prep_inputs(x, wq_a, bq_a, g_q, b_q, wq_b, bq_b, wkv_a, bkv_a, wkv_b, bkv_b,
                 wout, bout):
    bf = lambda a: np.ascontiguousarray(a).astype(BF)
    f32 = lambda a: np.ascontiguousarray(a).astype(np.float32)

    # rope tables (feature-major), one 64-row head block
    inv = 1.0 / (10000.0 ** (np.arange(0, DR, 2, dtype=np.float64) / DR))
    ang = np.arange(T, dtype=np.float64)[:, None] * inv[None, :]      # (T, 32)
    cos32 = np.cos(ang).T                                             # (32, T)
    sin32 = np.sin(ang).T
    cos64 = np.concatenate([cos32, cos32], axis=0)
    sin64 = np.concatenate([-sin32, sin32], axis=0)
    cosq = bf(np.concatenate([cos64, cos64], axis=0))    # (128, T): head pair
    sinqs = bf(np.concatenate([sin64, sin64], axis=0))
    perm = np.zeros((64, 64), dtype=np.float32)
    for m in range(64):
        perm[(m + 32) % 64, m] = 1.0   # swapped[m] = x[m+32 mod 64]
    perm128 = np.zeros((128, 128), dtype=np.float32)
    perm128[:64, :64] = perm
    perm128[64:, 64:] = perm
    perm = bf(perm)
    perm128 = bf(perm128)

    maskt = np.zeros((128, 4, 512), dtype=np.float32)
    for v in range(4):
        for p in range(128):
            maskt[p, v, v * 128 + p + 1:] = MASK_NEG

    wq_b3 = wq_b.reshape(H, DQK, QLR)
    wkv_b3 = wkv_b.reshape(H, DN + DV, KVLR)
    bq_b3 = bq_b.reshape(H, DQK)
    bkv_b3 = bkv_b.reshape(H, DN + DV)

    bkva_pad = np.zeros((640,), dtype=np.float32)
    bkva_pad[:KVLR + DR] = bkv_a

    shared = {
        "wqa": _chunk(bf(wq_a.T)).reshape(128, -1),
        "wkva": _chunk(bf(wkv_a.T)).reshape(128, -1),
        "cosq": cosq, "sinqs": sinqs, "perm64": perm, "perm128": perm128,
        "maskt": maskt,
        "bqa_t": f32(bq_a.reshape(8, 128).T),
        "gq_t": f32(g_q.reshape(8, 128).T),
        "bq_t": f32(b_q.reshape(8, 128).T),
        "bkva_t": f32(bkva_pad.reshape(5, 128).T),
        "dep": np.zeros((128, 16), np.float32),
    }

    # per-batch x (transposed, chunked); per-core we take the token quarter
    xt_by_batch = {b: _chunk(bf(x[b].T)) for b in range(B)}    # (128, 16, 2048)
    group_arrs = {}
    for g in range(HPG):  # 4 head groups
        hs = list(range(g * HPG, (g + 1) * HPG))
        wqbr_g = np.concatenate([wq_b3[h, :DR, :] for h in hs], axis=0)      # (256, QLR)
        wqbn_g = np.concatenate([wq_b3[h, DR:, :] for h in hs], axis=0)      # (512, QLR)
        wkvbk_g = np.concatenate([wkv_b3[h, :DN, :] for h in hs], axis=0)    # (512, KVLR)
        wkvbv_g = np.concatenate([wkv_b3[h, DN:, :] for h in hs], axis=0)    # (512, KVLR)
        wout_g = wout[:, g * HPG * DV:(g + 1) * HPG * DV]                    # (DIM, 512)
        group_arrs[g] = {
            "wqbn": _chunk(np.ascontiguousarray(wqbn_g.T * WQB_SCALE).astype(F8)).reshape(128, -1),
            "wqbr": _chunk(np.ascontiguousarray(wqbr_g.T * WQB_SCALE).astype(F8)).reshape(128, -1),
            "wkvbk": _chunk(bf(wkvbk_g.T)).reshape(128, -1),
            "wkvbv": _chunk(bf(wkvbv_g.T)).reshape(128, -1),
            "wout_l": _chunk(bf(np.ascontiguousarray(wout_g.T))).reshape(128, -1),
            "bqbn_t": f32(np.stack([bq_b3[h, DR:] for h in hs], axis=1)),    # (128, 4)
            "bqbr_t": f32(np.stack(
                [np.concatenate([bq_b3[hs[2 * hp], :DR], bq_b3[hs[2 * hp + 1], :DR]])
                 for hp in range(2)], axis=1)),                              # (128, 2)
            "bkvbk_t": f32(np.stack([bkv_b3[h, :DN] for h in hs], axis=1)),
            "bkvbv_row": f32(np.concatenate([bkv_b3[h, DN:] for h in hs])[None, :]),
        }
    in_maps = []
    for c in range(NCORES):
        b, g = divmod(c, HPG)
        m = dict(shared)
        qs = _ts(g, 512)
        m["xt"] = np.ascontiguousarray(xt_by_batch[b][:, :, qs]).reshape(128, -1)
        m["cosq_loc"] = np.ascontiguousarray(cosq[0:64, qs])
        m["sinqs_loc"] = np.ascontiguousarray(sinqs[0:64, qs])
        m.update(group_arrs[g])
        in_maps.append(m)
    return in_maps


def kernel(**inputs):
    inputs = {k: np.asarray(v) for k, v in inputs.items()}
    in_maps = _prep_inputs(**inputs)
    if "nc" not in _cached:
        _cached["nc"] = build_bass()
    res = run_bass_kernel_spmd(_cached["nc"], in_maps, core_ids=list(range(NCORES)))
    bout = inputs["bout"].astype(np.float64)
    out = np.zeros((B, T, DIM), dtype=np.float64)
    for c in range(NCORES):
        b = c // HPG
        out[b] += res.results[c]["outp"].astype(np.float64).T
    out += bout[None, None, :]
    return out.astype(np.float32)


if __name__ == "__main__":
    rng = np.random.default_rng(0)
    dummy = {
        "x": rng.standard_normal((B, T, DIM), dtype=np.float32),
        "wq_a": rng.standard_normal((QLR, DIM), dtype=np.float32) * 0.02,
        "bq_a": np.zeros(QLR, np.float32),
        "g_q": np.ones(QLR, np.float32),
        "b_q": np.zeros(QLR, np.float32),
        "wq_b": rng.standard_normal((H * DQK, QLR), dtype=np.float32) * 0.02,
        "bq_b": np.zeros(H * DQK, np.float32),
        "wkv_a": rng.standard_normal((KVLR + DR, DIM), dtype=np.float32) * 0.02,
        "bkv_a": np.zeros(KVLR + DR, np.float32),
        "wkv_b": rng.standard_normal((H * (DN + DV), KVLR), dtype=np.float32) * 0.02,
        "bkv_b": np.zeros(H * (DN + DV), np.float32),
        "wout": rng.standard_normal((DIM, DIM), dtype=np.float32) * 0.02,
        "bout": np.zeros(DIM, np.float32),
    }
    out = kernel(**dummy)
    print("out", out.shape, out.dtype, np.abs(out).max())


# revision 46
# speedup vs baseline: 1.0306x; 1.0306x over previous
"""MLA forward kernel for Trainium2, 8 NeuronCores.

Sharding: 2 batch groups x 4 head groups. Core c handles batch b=c//4 and
heads 4g..4g+3 where g=c%4. The LoRA down-projections (P1) are token-sharded
within each batch group: core (b, g) computes q/kv lora + layernorm + k-rope
rotation for token quarter g only, then two AllGathers (kv first, then q)
rebuild the full-T activations on every core. Attention and the partial
output projection stay head-sharded as before; the host sums 4 partials per
batch and adds the output bias.

Matmuls run in bf16 (fp32 PSUM accumulation), except the q up-projection
which runs in fp8e4m3 with DoubleRow pairing (two 128-row K planes per
instruction, 2x PE throughput): qln and wq_b are the two least
error-sensitive operands (LN bounds qln; softmax normalization absorbs
common-mode score error), measured output rel-err 8.7e-3 vs the 2e-2 gate.
Layout is feature-major (features on partitions, tokens on free dim).
RoPE rotate-half is a PE permutation matmul with the rotation signs folded
into the host-precomputed sin table. Causal softmax runs without max
subtraction; exp row-sums come from the scalar engine's accum_out.

Schedule notes: kv-lora runs before q-lora so its AllGather flies under the
q matmuls; P2 weight loads ride the scalar-DMA queue during P1; the P4
weight load is prefetched during attention. The "dep"/"dep_out" passthrough
tensors let the timing harness serially chain kernel executions.
"""
import sys

sys.path.insert(0, "/opt/trn_rl_repo")

import math
from contextlib import ExitStack

import numpy as np
import ml_dtypes

import concourse.bacc as bacc
import concourse.bass as bass
import concourse.tile as tile
from concourse import mybir
from concourse.bass_utils import run_bass_kernel_spmd
from concourse.masks import make_identity

F32 = mybir.dt.float32
BF16 = mybir.dt.bfloat16
FP8 = mybir.dt.float8e4
AF = mybir.ActivationFunctionType
ALU = mybir.AluOpType
DR_MODE = mybir.MatmulPerfMode.DoubleRow
BF = ml_dtypes.bfloat16
F8 = ml_dtypes.float8_e4m3
WQB_SCALE = 64.0   # wq_b is shipped as fp8 * WQB_SCALE; undone at PSUM readout

B, T, DIM = 2, 2048, 2048
H, QLR, KVLR = 16, 1024, 512
DN, DR, DV = 128, 64, 128
DQK = DN + DR
EPS = 1e-5
HPG = 4          # heads per group (per core)
NCORES = 8
SCALE = 1.0 / math.sqrt(DQK)
NT = T // 512    # 512-wide token tiles
NQT = T // 128   # 128-row query tiles
MASK_NEG = -1e30
RG = [[0, 1, 2, 3], [4, 5, 6, 7]]   # batch groups = AllGather replica groups

_cached = {}


def _ts(i, n):
    return slice(i * n, (i + 1) * n)


def build_bass():
    nc = bacc.Bacc("TRN2", target_bir_lowering=False, debug=False, num_devices=NCORES)

    inp = {}
    def di(name, shape, dt):
        inp[name] = nc.dram_tensor(name, list(shape), dt, kind="ExternalInput")
        return inp[name]

    # big tensors are flat (128, N) with per-partition-contiguous layout so
    # every load is 128 single-run descriptors
    di("xt", (128, 16 * 512), BF16)       # x[b].T quarter, chunked (p, cc, t)
    di("wqa", (128, 16 * QLR), BF16)      # wq_a.T chunked (p=c, cc, l)
    di("wkva", (128, 16 * (KVLR + DR)), BF16)
    di("wqbn", (128, 8 * HPG * DN), FP8)   # nope rows of wq_b (group), .T chunked by l
    di("wqbr", (128, 8 * HPG * DR), FP8)   # rope rows
    di("wkvbk", (128, 4 * HPG * DN), BF16)
    di("wkvbv", (128, 4 * HPG * DV), BF16)  # moving operand (p=lc, lc, hd)
    di("wout_l", (128, HPG * DIM), BF16)   # lhsT (p=hd within head, head, o)
    di("cosq", (128, T), BF16)             # [cos32;cos32] stacked twice (head pair)
    di("sinqs", (128, T), BF16)            # [-sin32;+sin32] stacked twice
    di("cosq_loc", (64, 512), BF16)        # this core's token-quarter columns
    di("sinqs_loc", (64, 512), BF16)
    di("perm64", (64, 64), BF16)           # rotate-half swap lhsT
    di("perm128", (128, 128), BF16)        # block-diag pair version
    di("maskt", (128, 4, 512), F32)        # additive causal masks, variant v=qt%4
    di("dep", (128, 16), F32)             # chain-dependency token (timing harness)
    di("bqa_t", (128, 8), F32)
    di("gq_t", (128, 8), F32)
    di("bq_t", (128, 8), F32)
    di("bqbn_t", (128, HPG), F32)
    di("bqbr_t", (128, 2), F32)
    di("bkva_t", (128, 5), F32)            # 576 rows chunked, last chunk rows 0:64
    di("bkvbk_t", (128, HPG), F32)
    di("bkvbv_row", (1, HPG * DV), F32)    # v bias as row (broadcast over partitions)

    outp = nc.dram_tensor("outp", [DIM, T], BF16, kind="ExternalOutput")
    dep_out = nc.dram_tensor("dep_out", [128, 16], F32, kind="ExternalOutput")

    with tile.TileContext(nc) as tc, ExitStack() as es:
        cst = es.enter_context(tc.tile_pool(name="cst", bufs=1))
        dram = es.enter_context(tc.tile_pool(name="dram", bufs=1, space="DRAM"))
        pD = es.enter_context(tc.tile_pool(name="pD", bufs=1))    # qln, kvl, kr (P1->P2)

        # ---- small constants (live whole kernel) ----
        ones_bf = cst.tile([128, 1], BF16)
        nc.vector.memset(ones_bf[:], 1.0)
        eps_t = cst.tile([1, 1], F32)
        nc.vector.memset(eps_t[:], EPS)
        # small constants ride the gpsimd queue so the sync queue starts with
        # the x tiles and the scalar queue with the P1 weights immediately
        perm = cst.tile([64, 64], BF16)
        nc.gpsimd.dma_start(out=perm[:], in_=inp["perm64"][:, :])
        cosq_loc = cst.tile([64, 512], BF16)
        nc.gpsimd.dma_start(out=cosq_loc[:], in_=inp["cosq_loc"][:, :])
        sinqs_loc = cst.tile([64, 512], BF16)
        nc.gpsimd.dma_start(out=sinqs_loc[:], in_=inp["sinqs_loc"][:, :])
        dep_t = cst.tile([128, 16], F32)
        nc.gpsimd.dma_start(out=dep_t[:], in_=inp["dep"][:, :])
        nc.gpsimd.dma_start(out=dep_out[:, :], in_=dep_t[:])
        bias_t = {}
        for nm, shape in [("bqa_t", (128, 8)), ("gq_t", (128, 8)), ("bq_t", (128, 8)),
                          ("bqbn_t", (128, HPG)), ("bqbr_t", (128, 2)),
                          ("bkva_t", (128, 5)), ("bkvbk_t", (128, HPG))]:
            bias_t[nm] = cst.tile(list(shape), F32, tag=nm, name=nm)
            nc.gpsimd.dma_start(out=bias_t[nm][:], in_=inp[nm][:, :])

        # ---- persistent full-T intermediates (gathered; P1 -> P2) ----
        qln = pD.tile([128, 8, T], FP8)       # layernormed q_lora (fp8), full T
        kvl = pD.tile([128, 4, T], BF16)      # kv_lora, full T
        kr = pD.tile([128, T], BF16)          # rotated k rope, duplicated halves

        # AllGather bounce buffers (DRAM); rank g's chunk = rows [128g:128(g+1))
        agkv_in = dram.tile([128, 5, 512], BF16)
        agkv_out = dram.tile([4 * 128, 5, 512], BF16)
        agq_in = dram.tile([128, 8, 512], FP8)
        agq_out = dram.tile([4 * 128, 8, 512], FP8)

        # P2 weights/tables: loaded on the scalar queue right after the P1
        # weights so P2 never waits on them
        pW = es.enter_context(tc.tile_pool(name="pW", bufs=1))
        wkvbv = pW.tile([128, 4, HPG * DV], BF16)
        vb_bc = pW.tile([128, HPG * DV], F32)
        wkvbk = pW.tile([128, 4, HPG * DN], BF16)
        cosq = pW.tile([128, T], BF16)     # two stacked 64-row head blocks
        sinqs = pW.tile([128, T], BF16)
        wqbn = pW.tile([128, 8, HPG * DN], FP8)
        wqbr = pW.tile([128, 8, HPG * DR], FP8)

        # ================= P1: LoRA projections (token quarter only) =========
        with tc.tile_pool(name="w1", bufs=1) as w1, \
             tc.tile_pool(name="xpa", bufs=1) as xpa, \
             tc.tile_pool(name="p1loc", bufs=1) as p1loc, \
             tc.tile_pool(name="p1e", bufs=2) as p1e, \
             tc.tile_pool(name="bcps", bufs=1, space="PSUM") as bcps_pool, \
             tc.tile_pool(name="p1ps", bufs=4, space="PSUM") as p1ps, \
             tc.tile_pool(name="stps", bufs=1, space="PSUM") as stps:
            # kv weights first: the kv-lora matmuls run first so their
            # AllGather can fly under the q-lora matmuls
            wkva = w1.tile([128, 16, KVLR + DR], BF16)
            nc.scalar.dma_start(out=wkva[:, 0:2, :], in_=inp["wkva"][:, 0:2 * (KVLR + DR)])
            nc.scalar.dma_start(out=wkva[:, 2:4, :],
                                in_=inp["wkva"][:, 2 * (KVLR + DR):4 * (KVLR + DR)])
            for c4 in range(1, 4):
                nc.scalar.dma_start(out=wkva[:, _ts(c4, 4), :],
                                    in_=inp["wkva"][:, c4 * 4 * (KVLR + DR):(c4 + 1) * 4 * (KVLR + DR)])
            wqa = w1.tile([128, 16, QLR], BF16)
            nc.scalar.dma_start(out=wqa[:], in_=inp["wqa"][:, :])
            # P2 weights follow on the same queue
            nc.scalar.dma_start(out=wkvbv[:, :, :], in_=inp["wkvbv"][:, :])
            nc.scalar.dma_start(out=vb_bc[:], in_=inp["bkvbv_row"][:, :].to_broadcast([128, HPG * DV]))
            nc.scalar.dma_start(out=wkvbk[:, :, :], in_=inp["wkvbk"][:, :])
            nc.scalar.dma_start(out=cosq[:], in_=inp["cosq"][:, :])
            nc.scalar.dma_start(out=sinqs[:], in_=inp["sinqs"][:, :])
            nc.scalar.dma_start(out=wqbn[:, :, :], in_=inp["wqbn"][:, :])
            nc.scalar.dma_start(out=wqbr[:, :, :], in_=inp["wqbr"][:, :])
            ones_row = cst.tile([1, 128], BF16, tag="ones_row", name="ones_row")
            nc.vector.memset(ones_row[:], 1.0)

            xtile = xpa.tile([128, 16, 512], BF16)
            nc.sync.dma_start(out=xtile[:, 0:2, :], in_=inp["xt"][:, 0:2 * 512])
            nc.sync.dma_start(out=xtile[:, 2:4, :], in_=inp["xt"][:, 2 * 512:4 * 512])
            for c4 in range(1, 4):
                nc.sync.dma_start(out=xtile[:, _ts(c4, 4), :],
                                  in_=inp["xt"][:, c4 * 4 * 512:(c4 + 1) * 4 * 512])

            # local (this quarter) staging tiles
            kvl_loc = p1loc.tile([128, 4, 512], BF16)
            kr_loc = p1loc.tile([64, 512], BF16)
            qloc = p1loc.tile([128, 8, 512], BF16)
            qloc8 = p1loc.tile([128, 8, 512], FP8)

            # ---- kv-lora in two PSUM waves (3 + 2 accumulators) ----
            for ocs in ([0, 1, 2], [3, 4]):
                pss = [(oc, p1ps.tile([128, 512], F32, tag="p1ps", name="ps"))
                       for oc in ocs]
                for cc in range(16):
                    for oc, ps in pss:
                        rows_n = 128 if oc < 4 else 64
                        nc.tensor.matmul(ps[:rows_n, :],
                                         wkva[:, cc, oc * 128:oc * 128 + rows_n],
                                         xtile[:, cc, :], start=(cc == 0), stop=(cc == 15))
                for oc, ps in pss:
                    if oc < 4:
                        nc.scalar.activation(out=kvl_loc[:, oc, :], in_=ps[:], func=AF.Identity,
                                             bias=bias_t["bkva_t"][:, oc:oc + 1])
                    else:
                        # decoupled k-rope: rotate locally (tables are this
                        # quarter's columns), pre-gather
                        kraw = p1e.tile([64, 512], BF16, tag="kraw", name="kraw")
                        nc.scalar.activation(out=kraw[:], in_=ps[:64, :], func=AF.Identity,
                                             bias=bias_t["bkva_t"][0:64, 4:5])
                        sw = p1ps.tile([128, 512], F32, tag="p1ps", name="sw")
                        nc.tensor.matmul(sw[:64, :], perm[:], kraw[:], start=True, stop=True)
                        ta = p1e.tile([64, 512], F32, tag="ropea", name="ta")
                        nc.vector.tensor_mul(ta[:], kraw[:], cosq_loc[:])
                        tb = p1e.tile([64, 512], F32, tag="ropeb", name="tb")
                        nc.vector.tensor_mul(tb[:], sw[:64, :], sinqs_loc[:])
                        nc.vector.tensor_add(kr_loc[:], ta[:], tb[:])
            # ship kv+kr chunk, kick AllGather 1
            nc.sync.dma_start(out=agkv_in[:, 0:4, :], in_=kvl_loc[:, :, :])
            nc.sync.dma_start(out=agkv_in[0:64, 4, :], in_=kr_loc[:])
            nc.gpsimd.collective_compute(
                "AllGather", ALU.bypass, replica_groups=RG,
                ins=[agkv_in.opt()], outs=[agkv_out.opt()])
            # kv/kr readback immediately after AG1 on the scalar queue (idle
            # once the weight loads drain) so P2's kv-side work never waits on
            # the q AllGather
            for tt in range(NT):
                rs = slice(tt * 128, (tt + 1) * 128)
                nc.scalar.dma_start(out=kvl[:, :, _ts(tt, 512)],
                                    in_=agkv_out[rs, 0:4, :])
                nc.scalar.dma_start(out=kr[0:64, _ts(tt, 512)],
                                    in_=agkv_out[tt * 128:tt * 128 + 64, 4, :])
                nc.scalar.dma_start(out=kr[64:128, _ts(tt, 512)],
                                    in_=agkv_out[tt * 128:tt * 128 + 64, 4, :])

            # ---- q-lora + LN (local quarter) ----
            stats = stps.tile([1, 1024], F32)

            for lc in range(8):
                ps = p1ps.tile([128, 512], F32, tag="p1ps")
                for cc in range(16):
                    nc.tensor.matmul(ps[:], wqa[:, cc, _ts(lc, 128)], xtile[:, cc, :],
                                     start=(cc == 0), stop=(cc == 15))
                nc.scalar.activation(out=qloc[:, lc, :], in_=ps[:], func=AF.Identity,
                                     bias=bias_t["bqa_t"][:, lc:lc + 1])
                sq = p1e.tile([128, 512], BF16, tag="sq")
                nc.vector.tensor_mul(sq[:], qloc[:, lc, :], qloc[:, lc, :])
                nc.tensor.matmul(stats[:, 0:512], ones_bf[:], qloc[:, lc, :],
                                 start=(lc == 0), stop=(lc == 7))
                nc.tensor.matmul(stats[:, 512:1024], ones_bf[:], sq[:],
                                 start=(lc == 0), stop=(lc == 7))
            r1 = p1e.tile([1, 512], F32, tag="r1")
            r2 = p1e.tile([1, 512], F32, tag="r2")
            mrow_bf = p1e.tile([1, 512], BF16, tag="mrow_bf")
            rrow_bf = p1e.tile([1, 512], BF16, tag="rrow_bf")
            mrow_f = p1e.tile([1, 512], F32, tag="mrow_f")
            nc.vector.tensor_scalar_mul(mrow_f[:], stats[0:1, 0:512], 1.0 / QLR)
            nc.vector.tensor_scalar_mul(r1[:], stats[0:1, 512:1024], 1.0 / QLR)
            nc.vector.tensor_mul(r2[:], mrow_f[:], mrow_f[:])
            nc.vector.tensor_sub(r1[:], r1[:], r2[:])          # var
            nc.scalar.activation(out=r2[:], in_=r1[:], func=AF.Sqrt, bias=eps_t[:])
            with nc.allow_low_precision(reason="LN row broadcast via PE"):
                nc.vector.reciprocal(out=rrow_bf[:], in_=r2[:])
                nc.vector.tensor_copy(out=mrow_bf[:], in_=mrow_f[:])
            bcps = bcps_pool.tile([128, 1024], F32, tag="bc", name="bc")
            nc.tensor.matmul(bcps[:, 0:512], ones_row[:], mrow_bf[:],
                             start=True, stop=True)
            nc.tensor.matmul(bcps[:, 512:1024], ones_row[:], rrow_bf[:],
                             start=True, stop=True)
            # LN apply in place on the local staging tile
            for lc in range(8):
                t1 = p1e.tile([128, 512], BF16, tag="lnt")
                nc.vector.tensor_sub(t1[:], qloc[:, lc, :], bcps[:, 0:512])
                nc.vector.tensor_mul(t1[:], t1[:], bcps[:, 512:1024])
                with nc.allow_low_precision(reason="fp8 qln for the up-projection"):
                    nc.scalar.activation(out=qloc8[:, lc, :], in_=t1[:], func=AF.Identity,
                                         scale=bias_t["gq_t"][:, lc:lc + 1],
                                         bias=bias_t["bq_t"][:, lc:lc + 1])
            # ship q chunk, kick AllGather 2
            nc.sync.dma_start(out=agq_in[:, :, :], in_=qloc8[:, :, :])
            nc.gpsimd.collective_compute(
                "AllGather", ALU.bypass, replica_groups=RG,
                ins=[agq_in.opt()], outs=[agq_out.opt()])

            # ---- qln readback (waits on AllGather 2) ----
            for tt in range(NT):
                rs = slice(tt * 128, (tt + 1) * 128)
                nc.sync.dma_start(out=qln[:, :, _ts(tt, 512)],
                                  in_=agq_out[rs, :, :])

        # ================= P2: up-projections + rope =================
        pG = es.enter_context(tc.tile_pool(name="pG", bufs=1))    # q/k/v heads (P2->P3)
        qnope = pG.tile([128, HPG, T], BF16)
        qrope = pG.tile([128, 2, T], BF16)   # head PAIR hp: h=2hp at rows 0:64, 2hp+1 at 64:128
        knope = pG.tile([128, HPG, T], BF16)
        vtm = pG.tile([128, NQT, HPG * DV], BF16)   # V token-major (k, kt, hd)
        perm2 = pG.tile([128, 128], BF16)
        nc.gpsimd.dma_start(out=perm2[:], in_=inp["perm128"][:, :])

        with tc.tile_pool(name="p2e", bufs=4) as p2e, \
             tc.tile_pool(name="p2ps", bufs=3, space="PSUM") as p2ps, \
             tc.tile_pool(name="p2ps64", bufs=2, space="PSUM") as p2ps64:

            def rope_block2(dst_ap, src_ap, ts):
                """dst = rotate_half(src), two stacked 64-row head blocks (128, 512)."""
                sw = p2ps64.tile([128, 512], F32, tag="swap", name="sw")
                nc.tensor.matmul(sw[:], perm2[:], src_ap, start=True, stop=True)
                ta = p2e.tile([128, 512], F32, tag="ropea", name="ta")
                nc.vector.tensor_mul(ta[:], src_ap, cosq[:, ts])
                tb = p2e.tile([128, 512], F32, tag="ropeb", name="tb")
                nc.vector.tensor_mul(tb[:], sw[:], sinqs[:, ts])
                nc.vector.tensor_add(dst_ap, ta[:], tb[:])

            # kv-side first: only depends on AllGather 1
            for kt in range(NQT):
                ps = p2ps.tile([128, 512], F32, tag="p2ps", name="ps")
                for lc in range(4):
                    nc.tensor.matmul(ps[:], kvl[:, lc, _ts(kt, 128)], wkvbv[:, lc, :],
                                     start=(lc == 0), stop=(lc == 3))
                nc.vector.tensor_add(vtm[:, kt, :], ps[:], vb_bc[:])
            for h in range(HPG):
                for tt in range(NT):
                    ts = _ts(tt, 512)
                    ps = p2ps.tile([128, 512], F32, tag="p2ps", name="ps")
                    for lc in range(4):
                        nc.tensor.matmul(ps[:], wkvbk[:, lc, _ts(h, DN)], kvl[:, lc, ts],
                                         start=(lc == 0), stop=(lc == 3))
                    nc.scalar.activation(out=knope[:, h, ts], in_=ps[:], func=AF.Identity,
                                         bias=bias_t["bkvbk_t"][:, h:h + 1])
            for tt in range(NT):
                for h in range(HPG):
                    ts = _ts(tt, 512)
                    # q nope
                    ps = p2ps.tile([128, 512], F32, tag="p2ps", name="ps")
                    for lp in range(4):
                        nc.tensor.matmul(ps[:], wqbn[:, 2 * lp:2 * lp + 2, _ts(h, DN)],
                                         qln[:, 2 * lp:2 * lp + 2, ts],
                                         perf_mode=DR_MODE,
                                         start=(lp == 0), stop=(lp == 3))
                    nc.scalar.activation(out=qnope[:, h, ts], in_=ps[:], func=AF.Identity,
                                         scale=1.0 / WQB_SCALE,
                                         bias=bias_t["bqbn_t"][:, h:h + 1])
                # q rope: two heads per matmul (full 128-wide PE)
                for hp in range(2):
                    ts = _ts(tt, 512)
                    ps2 = p2ps64.tile([128, 512], F32, tag="qr", name="ps2")
                    for lp in range(4):
                        nc.tensor.matmul(ps2[:], wqbr[:, 2 * lp:2 * lp + 2, _ts(hp, 2 * DR)],
                                         qln[:, 2 * lp:2 * lp + 2, ts],
                                         perf_mode=DR_MODE,
                                         start=(lp == 0), stop=(lp == 3))
                    qr_raw = p2e.tile([128, 512], BF16, tag="qr_raw", name="qr_raw")
                    nc.scalar.activation(out=qr_raw[:], in_=ps2[:], func=AF.Identity,
                                         scale=1.0 / WQB_SCALE,
                                         bias=bias_t["bqbr_t"][:, hp:hp + 1])
                    rope_block2(qrope[:, hp, ts], qr_raw[:], ts)

        # ================= P3: causal attention =================
        pI = es.enter_context(tc.tile_pool(name="pI", bufs=1))
        yt = pI.tile([128, HPG, T], BF16)           # attention out, feature-major
        wout_l = pI.tile([128, HPG, DIM], BF16)     # P4 weights, prefetched during P3
        nc.scalar.dma_start(out=wout_l[:, :, :], in_=inp["wout_l"][:, :])
        idb = pI.tile([128, 128], BF16)
        make_identity(nc, idb[:])

        with tc.tile_pool(name="amask", bufs=1) as amask, \
             tc.tile_pool(name="ap_s", bufs=3) as ap_s, \
             tc.tile_pool(name="ap_l", bufs=4) as ap_l, \
             tc.tile_pool(name="sps", bufs=3, space="PSUM") as spsp, \
             tc.tile_pool(name="ptps", bufs=2, space="PSUM") as ptps, \
             tc.tile_pool(name="yps", bufs=2, space="PSUM") as ypsp, \
             tc.tile_pool(name="ytps", bufs=1, space="PSUM") as ytpsp:
            maskt = amask.tile([128, 4, 512], F32)
            nc.sync.dma_start(out=maskt[:], in_=inp["maskt"][:, :, :])

            for h in range(HPG):
                for qt in range(NQT):
                    nkt = qt // 4 + 1
                    qs = _ts(qt, 128)
                    yps = ypsp.tile([128, 128], F32, tag="yacc", name="yps")
                    lpart = ap_l.tile([128, 4], F32, tag="lpart", name="lpart")
                    for kt in range(nkt):
                        # diagonal tile only covers its valid key width
                        diag = kt == qt // 4
                        w = (qt % 4 + 1) * 128 if diag else 512
                        nsub = qt % 4 + 1 if diag else 4
                        ks = slice(kt * 512, kt * 512 + w)
                        sps = spsp.tile([128, 512], F32, tag="sps", name="sps")
                        hb = 64 * (h % 2)
                        nc.tensor.matmul(sps[:, :w], qnope[:, h, qs], knope[:, h, ks],
                                         start=True, stop=False)
                        nc.tensor.matmul(sps[:, :w], qrope[hb:hb + 64, h // 2, qs],
                                         kr[hb:hb + 64, ks],
                                         start=False, stop=True)
                        if diag:
                            nc.vector.tensor_add(sps[:, :w], sps[:, :w],
                                                 maskt[:, qt % 4, :w])
                        pbf = ap_s.tile([128, 512], BF16, tag="pbf", name="pbf")
                        nc.scalar.activation(out=pbf[:, :w], in_=sps[:, :w], func=AF.Exp,
                                             scale=SCALE,
                                             accum_out=lpart[:, kt:kt + 1])
                        ptp = ptps.tile([128, 512], BF16, tag="ptp", name="ptp")
                        for i in range(nsub):
                            nc.tensor.transpose(ptp[:, _ts(i, 128)], pbf[:, _ts(i, 128)], idb[:])
                        pts = ap_s.tile([128, 512], BF16, tag="pts", name="pts")
                        nc.vector.tensor_copy(out=pts[:, :w], in_=ptp[:, :w])
                        for i in range(nsub):
                            nc.tensor.matmul(yps[:], pts[:, _ts(i, 128)],
                                             vtm[:, kt * 4 + i, _ts(h, DV)],
                                             start=(kt == 0 and i == 0),
                                             stop=(kt == nkt - 1 and i == nsub - 1))
                    lsum = ap_l.tile([128, 1], F32, tag="lsum", name="lsum")
                    nc.vector.tensor_reduce(lsum[:], lpart[:, 0:nkt],
                                            axis=mybir.AxisListType.X, op=ALU.add)
                    linv = ap_l.tile([128, 1], F32, tag="linv", name="linv")
                    nc.vector.reciprocal(out=linv[:], in_=lsum[:])
                    ytmb = ap_s.tile([128, 128], BF16, tag="ytmb", name="ytmb")
                    nc.vector.tensor_scalar_mul(ytmb[:], yps[:], linv[:])
                    ytp = ytpsp.tile([128, 128], BF16, tag="ytp", name="ytp")
                    nc.tensor.transpose(ytp[:], ytmb[:], idb[:])
                    nc.vector.tensor_copy(out=yt[:, h, qs], in_=ytp[:])

        # ================= P4: output projection (partial) =================
        with tc.tile_pool(name="p4e", bufs=4) as p4e, \
             tc.tile_pool(name="p4ps", bufs=4, space="PSUM") as p4ps:
            for oc in range(16):
                for tt in range(NT):
                    ts = _ts(tt, 512)
                    ps = p4ps.tile([128, 512], F32, tag="p4ps", name="ps")
                    for h in range(HPG):
                        nc.tensor.matmul(ps[:], wout_l[:, h, _ts(oc, 128)], yt[:, h, ts],
                                         start=(h == 0), stop=(h == HPG - 1))
                    ot = p4e.tile([128, 512], BF16, tag="ot", name="ot")
                    with nc.allow_low_precision(reason="bf16 output partials"):
                        nc.scalar.copy(out=ot[:], in_=ps[:])
                    nc.sync.dma_start(out=outp[_ts(oc, 128), ts], in_=ot[:])

    nc.compile()
    return nc


def _chunk(a, p=128):
    """(N, M) -> (p, N//p, M) with chunk index as middle dim."""
    n, m = a.shape
    return np.ascontiguousarray(a.reshape(n // p, p, m).swapaxes(0, 1))


def _prep_inputs(x, wq_a, bq_a, g_q, b_q, wq_b, bq_b, wkv_a, bkv_a, wkv_b, bkv_b,
                 wout, bout):
    bf = lambda a: np.ascontiguousarray(a).astype(BF)
    f32 = lambda a: np.ascontiguousarray(a).astype(np.float32)

    # rope tables (feature-major), one 64-row head block
    inv = 1.0 / (10000.0 ** (np.arange(0, DR, 2, dtype=np.float64) / DR))
    ang = np.arange(T, dtype=np.float64)[:, None] * inv[None, :]      # (T, 32)
    cos32 = np.cos(ang).T                                             # (32, T)
    sin32 = np.sin(ang).T
    cos64 = np.concatenate([cos32, cos32], axis=0)
    sin64 = np.concatenate([-sin32, sin32], axis=0)
    cosq = bf(np.concatenate([cos64, cos64], axis=0))    # (128, T): head pair
    sinqs = bf(np.concatenate([sin64, sin64], axis=0))
    perm = np.zeros((64, 64), dtype=np.float32)
    for m in range(64):
        perm[(m + 32) % 64, m] = 1.0   # swapped[m] = x[m+32 mod 64]
    perm128 = np.zeros((128, 128), dtype=np.float32)
    perm128[:64, :64] = perm
    perm128[64:, 64:] = perm
    perm = bf(perm)
    perm128 = bf(perm128)

    maskt = np.zeros((128, 4, 512), dtype=np.float32)
    for v in range(4):
        for p in range(128):
            maskt[p, v, v * 128 + p + 1:] = MASK_NEG

    wq_b3 = wq_b.reshape(H, DQK, QLR)
    wkv_b3 = wkv_b.reshape(H, DN + DV, KVLR)
    bq_b3 = bq_b.reshape(H, DQK)
    bkv_b3 = bkv_b.reshape(H, DN + DV)

    bkva_pad = np.zeros((640,), dtype=np.float32)
    bkva_pad[:KVLR + DR] = bkv_a

    shared = {
        "wqa": _chunk(bf(wq_a.T)).reshape(128, -1),
        "wkva": _chunk(bf(wkv_a.T)).reshape(128, -1),
        "cosq": cosq, "sinqs": sinqs, "perm64": perm, "perm128": perm128,
        "maskt": maskt,
        "bqa_t": f32(bq_a.reshape(8, 128).T),
        "gq_t": f32(g_q.reshape(8, 128).T),
        "bq_t": f32(b_q.reshape(8, 128).T),
        "bkva_t": f32(bkva_pad.reshape(5, 128).T),
        "dep": np.zeros((128, 16), np.float32),
    }

    # per-batch x (transposed, chunked); per-core we take the token quarter
    xt_by_batch = {b: _chunk(bf(x[b].T)) for b in range(B)}    # (128, 16, 2048)
    group_arrs = {}
    for g in range(HPG):  # 4 head groups
        hs = list(range(g * HPG, (g + 1) * HPG))
        wqbr_g = np.concatenate([wq_b3[h, :DR, :] for h in hs], axis=0)      # (256, QLR)
        wqbn_g = np.concatenate([wq_b3[h, DR:, :] for h in hs], axis=0)      # (512, QLR)
        wkvbk_g = np.concatenate([wkv_b3[h, :DN, :] for h in hs], axis=0)    # (512, KVLR)
        wkvbv_g = np.concatenate([wkv_b3[h, DN:, :] for h in hs], axis=0)    # (512, KVLR)
        wout_g = wout[:, g * HPG * DV:(g + 1) * HPG * DV]                    # (DIM, 512)
        group_arrs[g] = {
            "wqbn": _chunk(np.ascontiguousarray(wqbn_g.T * WQB_SCALE).astype(F8)).reshape(128, -1),
            "wqbr": _chunk(np.ascontiguousarray(wqbr_g.T * WQB_SCALE).astype(F8)).reshape(128, -1),
            "wkvbk": _chunk(bf(wkvbk_g.T)).reshape(128, -1),
            "wkvbv": _chunk(bf(wkvbv_g.T)).reshape(128, -1),
            "wout_l": _chunk(bf(np.ascontiguousarray(wout_g.T))).reshape(128, -1),
            "bqbn_t": f32(np.stack([bq_b3[h, DR:] for h in hs], axis=1)),    # (128, 4)
            "bqbr_t": f32(np.stack(
                [np.concatenate([bq_b3[hs[2 * hp], :DR], bq_b3[hs[2 * hp + 1], :DR]])
                 for hp in range(2)], axis=1)),                              # (128, 2)
            "bkvbk_t": f32(np.stack([bkv_b3[h, :DN] for h in hs], axis=1)),
            "bkvbv_row": f32(np.concatenate([bkv_b3[h, DN:] for h in hs])[None, :]),
        }
    in_maps = []
    for c in range(NCORES):
        b, g = divmod(c, HPG)
        m = dict(shared)
        qs = _ts(g, 512)
        m["xt"] = np.ascontiguousarray(xt_by_batch[b][:, :, qs]).reshape(128, -1)
        m["cosq_loc"] = np.ascontiguousarray(cosq[0:64, qs])
        m["sinqs_loc"] = np.ascontiguousarray(sinqs[0:64, qs])
        m.update(group_arrs[g])
        in_maps.append(m)
    return in_maps


def kernel(**inputs):
    inputs = {k: np.asarray(v) for k, v in inputs.items()}
    in_maps = _prep_inputs(**inputs)
    if "nc" not in _cached:
        _cached["nc"] = build_bass()
    res = run_bass_kernel_spmd(_cached["nc"], in_maps, core_ids=list(range(NCORES)))
    bout = inputs["bout"].astype(np.float64)
    out = np.zeros((B, T, DIM), dtype=np.float64)
    for c in range(NCORES):
        b = c // HPG
        out[b] += res.results[c]["outp"].astype(np.float64).T
    out += bout[None, None, :]
    return out.astype(np.float32)


if __name__ == "__main__":
    rng = np.random.default_rng(0)
    dummy = {
        "x": rng.standard_normal((B, T, DIM), dtype=np.float32),
        "wq_a": rng.standard_normal((QLR, DIM), dtype=np.float32) * 0.02,
        "bq_a": np.zeros(QLR, np.float32),
        "g_q": np.ones(QLR, np.float32),
        "b_q": np.zeros(QLR, np.float32),
        "wq_b": rng.standard_normal((H * DQK, QLR), dtype=np.float32) * 0.02,
        "bq_b": np.zeros(H * DQK, np.float32),
        "wkv_a": rng.standard_normal((KVLR + DR, DIM), dtype=np.float32) * 0.02,
        "bkv_a": np.zeros(KVLR + DR, np.float32),
        "wkv_b": rng.standard_normal((H * (DN + DV), KVLR), dtype=np.float32) * 0.02,
        "bkv_b": np.zeros(H * (DN + DV), np.float32),
        "wout": rng.standard_normal((DIM, DIM), dtype=np.float32) * 0.02,
        "bout": np.zeros(DIM, np.float32),
    }
    out = kernel(**dummy)
    print("out", out.shape, out.dtype, np.abs(out).max())
